# revision 2
# baseline (speedup 1.0000x reference)
"""Trainium2 Bass kernel for nn_HardAttention (L == S branch).

Math (from the reference, with L == S so the one-hot gather is identity):
    mix      = context                                    # [B, L, D]
    combined = concat(mix, output, axis=2)                # [B, L, 2D]
    out      = tanh(combined @ W.T + b)                   # [B, L, D]
    attn     = broadcast identity one-hot                 # [B, L, S], constant

Strategy: pure data parallel over the 8 NeuronCores — flatten B*L = 16384
tokens, 2048 tokens per core. On each core compute a [2048, 1024] x
[1024, 512] GEMM with fused tanh(+bias).

Device layout: the contraction dim (d = 1024) must live on SBUF partitions
for the PE, so the host pre-transposes the activations to x^T [1024, T] and
the weight to W^T, and the kernel computes y^T [512 out-chan, T tokens]
(W chunks stationary, tokens streaming). The host transposes y^T back.
fp32 data is fed to the PE as float32r (single-pass fp32 matmul).
"""

import numpy as np

B, L, S, D = 4, 4096, 4096, 512
CORES = 8
T = (B * L) // CORES      # 2048 tokens per core
K = 2 * D                 # 1024 contraction dim
P = 128                   # partitions
KT = K // P               # 8 contraction tiles
OC = D // P               # 4 output-channel tiles
NT = 512                  # moving free dim per matmul (fp32 max)
TCH = 1024                # token chunk per DMA
NCH = T // TCH            # chunks per core
NTG = TCH // NT           # matmul token groups per chunk

_COMPILED = None


def _build():
    import concourse.bacc as bacc
    import concourse.mybir as mybir
    from concourse import bass
    from concourse.tile import TileContext

    f32 = mybir.dt.float32
    f32r = mybir.dt.float32r

    nc = bacc.Bacc(
        "TRN2",
        target_bir_lowering=False,
        debug=False,
        enable_asserts=False,
        num_devices=CORES,
    )

    xt = nc.declare_dram_parameter("xt", [NCH * KT * P, TCH], f32r, isOutput=False)
    wt = nc.declare_dram_parameter("wt", [P, KT * D], f32r, isOutput=False)
    bc = nc.declare_dram_parameter("bc", [P, OC], f32, isOutput=False)
    yt = nc.declare_dram_parameter("yt", [D, T], f32, isOutput=True)

    with TileContext(nc) as tc:
        with (
            tc.tile_pool(name="const", bufs=1) as cp,
            tc.tile_pool(name="xp", bufs=4) as xp,
            tc.tile_pool(name="yp", bufs=4) as yp,
            tc.tile_pool(name="pp", bufs=8, space=bass.MemorySpace.PSUM) as pp,
        ):
            b_t = cp.tile([P, OC], f32, tag="bias", name="b_t")
            nc.sync.dma_start(b_t[:], bc[:])
            w_t = cp.tile([P, KT * D], f32r, tag="w", name="w_t")
            nc.sync.dma_start(w_t[:], wt[:])

            for ch in range(NCH):
                x_ts = []
                for k in range(KT):
                    x_t = xp.tile([P, TCH], f32r, tag="x", name=f"x_{ch}_{k}")
                    r0 = (ch * KT + k) * P
                    nc.sync.dma_start(x_t[:], xt[r0 : r0 + P, :])
                    x_ts.append(x_t)

                ps = [[pp.tile([P, NT], f32, tag="ps", name=f"ps_{ch}_{oc}_{tg}")
                       for tg in range(NTG)] for oc in range(OC)]
                for k in range(KT):
                    for oc in range(OC):
                        lhsT = w_t[:, k * D + oc * P : k * D + (oc + 1) * P]
                        for tg in range(NTG):
                            nc.tensor.matmul(
                                ps[oc][tg][:],
                                lhsT,
                                x_ts[k][:, tg * NT : (tg + 1) * NT],
                                start=(k == 0),
                                stop=(k == KT - 1),
                            )

                for oc in range(OC):
                    y_t = yp.tile([P, TCH], f32, tag="y", name=f"y_{ch}_{oc}")
                    for tg in range(NTG):
                        nc.scalar.activation(
                            y_t[:, tg * NT : (tg + 1) * NT],
                            ps[oc][tg][:],
                            mybir.ActivationFunctionType.Tanh,
                            bias=b_t[:, oc : oc + 1],
                        )
                    nc.sync.dma_start(
                        yt[oc * P : (oc + 1) * P, ch * TCH : (ch + 1) * TCH],
                        y_t[:],
                    )

    nc.compile()
    return nc


def _get_compiled():
    global _COMPILED
    if _COMPILED is None:
        _COMPILED = _build()
    return _COMPILED


def _prep_inputs(output, context, W, b):
    out_f = np.asarray(output, dtype=np.float32).reshape(B * L, D)
    ctx_f = np.asarray(context, dtype=np.float32).reshape(B * L, D)
    comb = np.concatenate([ctx_f, out_f], axis=1)          # [16384, 1024]
    xt_all = np.ascontiguousarray(comb.T)                  # [1024, 16384]

    # wt[p, k*D + o] = W[o, k*P + p]
    w_host = np.ascontiguousarray(
        np.asarray(W, dtype=np.float32).T.reshape(KT, P, D).transpose(1, 0, 2)
    ).reshape(P, KT * D)
    bc_host = np.ascontiguousarray(
        np.asarray(b, dtype=np.float32).reshape(OC, P).T
    )

    in_maps = []
    for c in range(CORES):
        xt_c = xt_all[:, c * T : (c + 1) * T]              # [1024, 2048]
        # -> [NCH, KT, P, TCH] chunk-major contiguous blocks
        xt_c = np.ascontiguousarray(
            xt_c.reshape(KT, P, NCH, TCH).transpose(2, 0, 1, 3)
        ).reshape(NCH * KT * P, TCH)
        in_maps.append({"xt": xt_c, "wt": w_host, "bc": bc_host})
    return in_maps


def _run(output, context, W, b, trace=False, trace_cores=None, tmpdir=None):
    from concourse.bass_utils import run_bass_kernel_spmd

    nc = _get_compiled()
    in_maps = _prep_inputs(output, context, W, b)
    res = run_bass_kernel_spmd(
        nc,
        in_maps,
        list(range(CORES)),
        trace=trace,
        trace_cores=trace_cores,
        tmpdir=tmpdir,
    )
    y = np.empty((B * L, D), dtype=np.float32)
    for c in range(CORES):
        y[c * T : (c + 1) * T] = res.results[c]["yt"].T
    return y.reshape(B, L, D), res


def kernel(output, context, W, b, di=None):
    y, _ = _run(output, context, W, b)
    attn = np.broadcast_to(
        np.eye(L, S, dtype=np.float32)[None], (B, L, S)
    )
    return y, attn


# revision 3
# speedup vs baseline: 1.2874x; 1.2874x over previous
"""Trainium2 Bass kernel for nn_HardAttention (L == S branch).

Math (from the reference, with L == S so the one-hot gather is identity):
    mix      = context                                    # [B, L, D]
    combined = concat(mix, output, axis=2)                # [B, L, 2D]
    out      = tanh(combined @ W.T + b)                   # [B, L, D]
    attn     = broadcast identity one-hot                 # [B, L, S], constant

Strategy: pure data parallel over the 8 NeuronCores — flatten B*L = 16384
tokens, 2048 tokens per core. On each core compute a [2048, 1024] x
[1024, 512] GEMM with fused tanh(+bias).

Device layout: the contraction dim (d = 1024) must live on SBUF partitions
for the PE, so the host pre-transposes the activations to x^T [1024, T] and
the weight to W^T, and the kernel computes y^T [512 out-chan, T tokens]
(W chunks stationary, tokens streaming). The host transposes y^T back.
Activations/weights are fed in fp16 (fp32 PSUM accumulation): full-rate PE,
half the input DMA bytes, FWL fast weight loads.
"""

import numpy as np

B, L, S, D = 4, 4096, 4096, 512
CORES = 8
T = (B * L) // CORES      # 2048 tokens per core
K = 2 * D                 # 1024 contraction dim
P = 128                   # partitions
KT = K // P               # 8 contraction tiles
OC = D // P               # 4 output-channel tiles
NT = 512                  # moving free dim per matmul
TCH = 1024                # token chunk per psum group
NCH = T // TCH            # chunks per core
NTG = TCH // NT           # matmul token groups per chunk
XDMA = 4                  # input DMA count (1 MB each)
WARM = 18                 # PE warm-up matmuls

_COMPILED = None


def _build():
    import concourse.bacc as bacc
    import concourse.mybir as mybir
    from concourse import bass
    from concourse.tile import TileContext

    f32 = mybir.dt.float32
    f16 = mybir.dt.float16

    nc = bacc.Bacc(
        "TRN2",
        target_bir_lowering=False,
        debug=False,
        enable_asserts=False,
        num_devices=CORES,
    )

    xt = nc.declare_dram_parameter("xt", [P, KT * T], f16, isOutput=False)
    wt = nc.declare_dram_parameter("wt", [P, KT * D], f16, isOutput=False)
    bc = nc.declare_dram_parameter("bc", [P, OC], f32, isOutput=False)
    yt = nc.declare_dram_parameter("yt", [D, T], f32, isOutput=True)

    with TileContext(nc) as tc:
        with (
            tc.tile_pool(name="const", bufs=1) as cp,
            tc.tile_pool(name="xp", bufs=1) as xp,
            tc.tile_pool(name="yp", bufs=4) as yp,
            tc.tile_pool(name="pp", bufs=8, space=bass.MemorySpace.PSUM) as pp,
        ):
            b_t = cp.tile([P, OC], f32, tag="bias", name="b_t")
            nc.sync.dma_start(b_t[:], bc[:])
            w_t = cp.tile([P, KT * D], f16, tag="w", name="w_t")
            nc.sync.dma_start(w_t[:], wt[:])

            x_all = xp.tile([P, KT * T], f16, tag="x", name="x_all")
            step = (KT * T) // XDMA
            for j in range(XDMA):
                nc.sync.dma_start(
                    x_all[:, j * step : (j + 1) * step],
                    xt[:, j * step : (j + 1) * step],
                )

            # PE warm-up: junk matmuls to lift the HAM clock gate while the
            # first input DMAs are still in flight.
            junk = cp.tile([P, P], f16, tag="junk", name="junk")
            nc.gpsimd.memset(junk[:], 0.0)
            ps_w = pp.tile([P, NT], f32, tag="ps", name="ps_warm")
            for _ in range(WARM):
                nc.tensor.matmul(
                    ps_w[:, :P], junk[:], junk[:], start=True, stop=True
                )

            for ch in range(NCH):
                ps = [[pp.tile([P, NT], f32, tag="ps", name=f"ps_{ch}_{oc}_{tg}")
                       for tg in range(NTG)] for oc in range(OC)]
                for k in range(KT):
                    for oc in range(OC):
                        lhsT = w_t[:, k * D + oc * P : k * D + (oc + 1) * P]
                        for tg in range(NTG):
                            nc.tensor.matmul(
                                ps[oc][tg][:],
                                lhsT,
                                x_all[:, k * T + ch * TCH + tg * NT :
                                         k * T + ch * TCH + (tg + 1) * NT],
                                start=(k == 0),
                                stop=(k == KT - 1),
                            )

                for oc in range(OC):
                    y_t = yp.tile([P, TCH], f32, tag="y", name=f"y_{ch}_{oc}")
                    for tg in range(NTG):
                        nc.scalar.activation(
                            y_t[:, tg * NT : (tg + 1) * NT],
                            ps[oc][tg][:],
                            mybir.ActivationFunctionType.Tanh,
                            bias=b_t[:, oc : oc + 1],
                        )
                    nc.scalar.dma_start(
                        yt[oc * P : (oc + 1) * P, ch * TCH : (ch + 1) * TCH],
                        y_t[:],
                    )

    nc.compile()
    return nc


def _get_compiled():
    global _COMPILED
    if _COMPILED is None:
        _COMPILED = _build()
    return _COMPILED


def _prep_inputs(output, context, W, b):
    out_f = np.asarray(output, dtype=np.float32).reshape(B * L, D)
    ctx_f = np.asarray(context, dtype=np.float32).reshape(B * L, D)
    comb = np.concatenate([ctx_f, out_f], axis=1)          # [16384, 1024]
    xt_all = np.ascontiguousarray(comb.T.astype(np.float16))  # [1024, 16384]

    # wt[p, k*D + o] = W[o, k*P + p]
    w_host = np.ascontiguousarray(
        np.asarray(W, dtype=np.float32).T.reshape(KT, P, D).transpose(1, 0, 2)
    ).reshape(P, KT * D).astype(np.float16)
    bc_host = np.ascontiguousarray(
        np.asarray(b, dtype=np.float32).reshape(OC, P).T
    )

    in_maps = []
    for c in range(CORES):
        xt_c = xt_all[:, c * T : (c + 1) * T]              # [1024, 2048]
        # -> [P, KT*T]: xt2[p, k*T + t] = x^T[k*128+p, t]
        xt_c = np.ascontiguousarray(
            xt_c.reshape(KT, P, T).transpose(1, 0, 2)
        ).reshape(P, KT * T)
        in_maps.append({"xt": xt_c, "wt": w_host, "bc": bc_host})
    return in_maps


def _run(output, context, W, b, trace=False, trace_cores=None, tmpdir=None):
    from concourse.bass_utils import run_bass_kernel_spmd

    nc = _get_compiled()
    in_maps = _prep_inputs(output, context, W, b)
    res = run_bass_kernel_spmd(
        nc,
        in_maps,
        list(range(CORES)),
        trace=trace,
        trace_cores=trace_cores,
        tmpdir=tmpdir,
    )
    y = np.empty((B * L, D), dtype=np.float32)
    for c in range(CORES):
        y[c * T : (c + 1) * T] = res.results[c]["yt"].T
    return y.reshape(B, L, D), res


def kernel(output, context, W, b, di=None):
    y, _ = _run(output, context, W, b)
    attn = np.broadcast_to(
        np.eye(L, S, dtype=np.float32)[None], (B, L, S)
    )
    return y, attn
